# revision 50
# baseline (speedup 1.0000x reference)
"""MACE-style GNN message passing on 8 Trainium2 NeuronCores.

Only the l=0 (scalar) channel of the reference reaches the output, so the
network collapses algebraically: per edge, the radial MLP's last hidden
t3 (64) is dotted with a per-(sender-species, receiver-species) vector
Gamma[s,z] = W4_0 @ (hu[s] * delta[z]), where hu = w_embed@w_up and
delta[z] folds w_lin[0], w_sym[0], w_lin2[0] and w_readout.  Node energy
is then ae[z]+beta[z] + (1/16) * scatter_sum(eps_e).

v2 device pipeline (vs v1): geometry split into independent 16-subtile
units with private tiles (U0/U2/U4 on DVE, U1/U3 on the otherwise-idle
Pool engine); act tables preloaded with dummy calls; warm-up matmuls
gated on an early small DMA; silu1+silu2 fused into one [128,1024] ACT
over a 2-bank PSUM arena; L1 uses [16,128] block-diag weights (LDW 16);
per-edge scalar eps = sum_h t3*Gamma computed on DVE so the scatter
matmuls have N=1 and MSG is just [128, 8]; epilogue is one fused STT.

Sharding: receivers range-partitioned via degree-balanced greedy packing
into 64 128-node tiles (8 tiles/core); per (core, node-tile) edge groups
padded to a uniform SEG subtiles of 128 so all cores run one SPMD
program.  Edges with r >= r_max are dropped on host; pad slots are
masked by zero one-hot rows and zero Gamma rows.
"""

import sys
import numpy as np

sys.path.insert(0, "/opt/trn_rl_repo")

import ml_dtypes

BF16 = ml_dtypes.bfloat16

R_MAX = 5.0
AVG = 16.0
N_NODES = 8000
Z = 10
K = 128
NB = 8
NCORES = 8
NPC = N_NODES // NCORES       # nodes per core (1000)
NT = 8                        # node tiles per core (128 nodes each)

SIN_DIRECT = False     # ACT Sin table cannot handle args beyond ~pi
TRACE = False
LAST_RESULTS = None

_prog_cache = {}


def _build_program(SEG):
    """SPMD Bass program; SEG = 128-edge subtiles per 128-node tile."""
    from concourse import bass, bacc, mybir
    from concourse.tile import TileContext
    from contextlib import ExitStack

    f32 = mybir.dt.float32
    bf16 = mybir.dt.bfloat16
    i32 = mybir.dt.int32
    AF = mybir.ActivationFunctionType
    OP = mybir.AluOpType
    PSUM = bass.MemorySpace.PSUM

    S = NT * SEG              # total subtiles per core
    NBLK = S // 8             # 1024-edge blocks
    SA_W = min(32, S)         # subtiles in the early [r,1/r] DMA

    nc = bacc.Bacc(None, target_bir_lowering=False)

    cf_d = nc.dram_tensor("constf", [128, 26], f32, kind="ExternalInput")
    vea_d = nc.dram_tensor("vea", [128, 2 * SA_W], f32, kind="ExternalInput")
    ceb_d = nc.dram_tensor("ceb", [128, 128], bf16, kind="ExternalInput")
    veb_d = nc.dram_tensor("veb", [128, 2 * (S - SA_W)], f32,
                           kind="ExternalInput")
    crb_d = nc.dram_tensor("crb", [128, 768], bf16, kind="ExternalInput")
    g_d = nc.dram_tensor("gtab", [NBLK, 128, 512], bf16, kind="ExternalInput")
    ohr_d = nc.dram_tensor("ohr", [NBLK, 128, 1024], bf16,
                           kind="ExternalInput")
    out_d = nc.dram_tensor("out", [128, 8], f32, kind="ExternalOutput")

    with TileContext(nc) as tc:
        with ExitStack() as stack:
            # one pool per DMA'd tensor: readers of a pool's tile appear to
            # wait on ALL outstanding DMAs into that pool, so sharing a pool
            # serializes consumers behind the slowest DMA
            cpf = stack.enter_context(tc.tile_pool(name="cpf", bufs=1))
            cpe = stack.enter_context(tc.tile_pool(name="cpe", bufs=1))
            cpr = stack.enter_context(tc.tile_pool(name="cpr", bufs=1))
            vpa = stack.enter_context(tc.tile_pool(name="vpa", bufs=1))
            vpb = stack.enter_context(tc.tile_pool(name="vpb", bufs=1))
            cp = stack.enter_context(tc.tile_pool(name="const", bufs=1))
            geo = stack.enter_context(tc.tile_pool(name="geo", bufs=1))
            efsp = stack.enter_context(tc.tile_pool(name="efsp", bufs=3))
            gp = stack.enter_context(tc.tile_pool(name="gp", bufs=5))
            ohp = stack.enter_context(tc.tile_pool(name="ohp", bufs=5))
            ttp = stack.enter_context(tc.tile_pool(name="ttp", bufs=3))
            t3p = stack.enter_context(tc.tile_pool(name="t3p", bufs=3))
            qp = stack.enter_context(tc.tile_pool(name="qp", bufs=3))
            epp = stack.enter_context(tc.tile_pool(name="epp", bufs=3))
            outp = stack.enter_context(tc.tile_pool(name="outp", bufs=1))
            pefp = stack.enter_context(tc.tile_pool(name="pefp", bufs=2,
                                                    space=PSUM))
            par = stack.enter_context(tc.tile_pool(name="par", bufs=2,
                                                   space=PSUM))
            pq3 = stack.enter_context(tc.tile_pool(name="pq3", bufs=1,
                                                   space=PSUM))
            pmsg = stack.enter_context(tc.tile_pool(name="pmsg", bufs=1,
                                                    space=PSUM))

            # ---- constants: smallest / most critical DMAs first ----
            CTF = cpf.tile([128, 26], f32)
            nc.sync.dma_start(CTF[:], cf_d[:], single_packet=True)
            VEA = vpa.tile([128, 2 * SA_W], f32, name="VEA")
            nc.sync.dma_start(VEA[:], vea_d[:])
            CEB = cpe.tile([128, 128], bf16)
            nc.sync.dma_start(CEB[:], ceb_d[:])
            VEB = vpb.tile([128, 2 * (S - SA_W)], f32, name="VEB")
            nc.sync.dma_start(VEB[:], veb_d[:])
            CRB = cpr.tile([128, 768], bf16)
            nc.sync.dma_start(CRB[:], crb_d[:])

            CB8 = CTF[:, 0:8]
            CNODE = CTF[:, 8:16]
            ONEI = CTF[:, 16:17].bitcast(i32)
            MAGIC = CTF[:, 17:18].bitcast(i32)
            MAGICF = float(0x5F3759DF)
            CCOL = {v: CTF[:, 18 + k:19 + k] for k, v in enumerate(
                [0.5, 1.5, 15.0, 21.0, 35.0, 1.0, 1.0 / R_MAX, MAGICF])}
            I128 = CEB[:, 0:128]
            # W1P duplicated in both partition halves so the lhsT base
            # partition can match the rhs (efs half) base partition
            W1P = [[CRB[ro:ro + 64, 128 * j:128 * j + 128]
                    for j in range(4)] for ro in (0, 64)]
            W2BD = CRB[:, 512:640]
            W3XY = CRB[:, 640:768]

            tc.strict_bb_all_engine_barrier()

            # ---- ACT table preload: dummy Sin + Silu on scratch ----
            SCR = cp.tile([128, 1], f32)
            nc.gpsimd.memset(SCR[:], 0.25)
            DS = cp.tile([128, 1], f32)
            nc.scalar.activation(DS[:], SCR[:], AF.Sin, scale=1.0)
            nc.scalar.activation(DS[:], SCR[:], AF.Silu)

            # ---- PE pstate warm-up: small matmuls gated on CEB only ----
            WUP = pq3.tile([128, 512], f32, tag="q3")
            for _ in range(5):
                nc.tensor.matmul(WUP[:, 0:128], I128, I128,
                                 start=True, stop=True, skip_group_check=True)

            # ---- geometry, all on DVE.  Host supplies per-edge [r, 1/r]
            # (edge lengths; already computed host-side for the r<R filter).
            # Device computes the cutoff envelope, bessel phases, sin and
            # the ef features.  (Pool proved ~5x slower per op on hw.)
            V = nc.vector
            P = nc.gpsimd
            SC = geo.tile([128, 4 * S], f32, name="SC")

            def rsl(s0, s1):
                """(r, inv_r) slices for subtile-cols [s0, s1)."""
                if s1 <= SA_W:
                    return (VEA[:, s0:s1], VEA[:, SA_W + s0:SA_W + s1])
                o = s0 - SA_W
                return (VEB[:, o:o + (s1 - s0)],
                        VEB[:, (S - SA_W) + o:(S - SA_W) + o + (s1 - s0)])

            def emit_scalars(s0, s1):
                """cutoff envelope env(r)/r for subtile-cols [s0, s1)."""
                def sl(i):
                    return SC[:, i * S + s0:i * S + s1]

                x, u1, u2, wv = (sl(i) for i in range(4))
                r_, ir_ = rsl(s0, s1)
                V.tensor_scalar(x, r_, 1.0 / R_MAX, None, OP.mult)
                V.tensor_tensor(u1, x, x, OP.mult)
                V.tensor_tensor(u1, u1, u1, OP.mult)
                V.tensor_tensor(u1, u1, x, OP.mult)      # x^5
                V.tensor_scalar(u2, x, -15.0, 35.0, OP.mult, OP.add)
                V.tensor_tensor(u2, u2, x, OP.mult)
                V.scalar_tensor_tensor(u1, u2, -21.0, u1, OP.add, OP.mult)
                V.scalar_tensor_tensor(wv, u1, 1.0, ir_, OP.add, OP.mult)
                # wv = env(r)/r  (sqrt(2/R) folded into W1)

            # bessel groups: 16-subtile strides (2 blocks per group, so the
            # ef transpose can be one [128,128] DMA-xbar per group)
            GBOUND = list(range(0, S, 16)) + [S]
            NG = len(GBOUND) - 1
            GT = {}

            def emit_bessel(g):
                s0, s1 = GBOUND[g], GBOUND[g + 1]
                w = s1 - s0
                t = {
                    "TH": geo.tile([128, 8 * w], f32, name=f"TH{g}"),
                    "SH": geo.tile([128, 8 * w], f32, name=f"SH{g}"),
                    "EFB": geo.tile([128, 8 * w], bf16, name=f"EFB{g}"),
                }
                GT[g] = t
                r_, _ = rsl(s0, s1)
                wenv = SC[:, 3 * S + s0:3 * S + s1]
                V.tensor_tensor(
                    t["TH"][:].rearrange("p (s b) -> p s b", b=8),
                    CB8.unsqueeze(1).broadcast_to([128, w, 8]),
                    r_.unsqueeze(2).broadcast_to([128, w, 8]),
                    OP.mult)
                ki = geo.tile([128, 8 * w], i32, name=f"KI{g}")
                kf = geo.tile([128, 8 * w], f32, name=f"KF{g}")
                sa = geo.tile([128, 8 * w], f32, name=f"SA{g}")
                V.tensor_copy(ki[:], t["TH"][:])
                V.tensor_copy(kf[:], ki[:])
                V.tensor_tensor(sa[:], t["TH"][:], kf[:], OP.subtract)
                nc.scalar.activation(t["SH"][:], sa[:], AF.Sin,
                                     scale=float(2 * np.pi))
                V.tensor_tensor(
                    t["EFB"][:].rearrange("p (s b) -> p s b", b=8),
                    t["SH"][:].rearrange("p (s b) -> p s b", b=8),
                    wenv.unsqueeze(2).broadcast_to([128, w, 8]),
                    OP.mult)

            def efb_col(i):
                """EFB access for block i: (group tile, col offset)."""
                g = (8 * i) // 16
                off = 8 * (8 * i - GBOUND[g])
                return GT[g]["EFB"], off

            # chains: [0:SA_W] pre, rest @iter0; bessel g0 pre, g @iter g-1
            emit_scalars(0, min(SA_W, S))
            emit_bessel(0)

            tail = {}

            def add_tail(it, fn):
                tail.setdefault(it, []).append(fn)

            for g in range(1, NG):
                add_tail(g - 1, (lambda gg: lambda: emit_bessel(gg))(g))
            if S > SA_W:
                # after bessel1 in tail[0]: bessel1 only needs chain A
                add_tail(0, lambda: emit_scalars(SA_W, S))

            # ---- software-pipelined block loop ----
            # stage skew: efT(i) -> L1(i-1) -> L2(i-2) + fused silu12
            #   -> L3T(i-3)+silu3+qss -> scatter(i-4)
            MSG = pmsg.tile([128, 512], f32, tag="msg")
            efs = {}
            tts = {}   # per-iter arena: [0:512]=t1(i-1), [512:1024]=t2(i-2)
            t3s = {}
            qss = {}
            gts = {}
            ohrs = {}
            NITER = NBLK + 4
            for i in range(NITER):
                if i < NBLK:
                    gts[i] = gp.tile([128, 512], bf16, tag="gt", name="gt")
                    nc.sync.dma_start(gts[i][:], g_d[i])
                    ohrs[i] = ohp.tile([128, 1024], bf16, tag="ohr",
                                       name="ohrt")
                    nc.sync.dma_start(ohrs[i][:], ohr_d[i])

                # stage 1: ef transpose for a 2-block group via DMA xbar
                # (frees PE + DVE); odd leftover group falls back to PE
                if i < NBLK and i % 2 == 0:
                    g = i // 2
                    gw = GBOUND[g + 1] - GBOUND[g]
                    efs[g] = efsp.tile([8 * gw, 128], bf16, tag="efs",
                                       name="efs")
                    if 8 * gw == 128:
                        nc.sync.dma_start_transpose(efs[g][:],
                                                    GT[g]["EFB"][:])
                    else:
                        pef = pefp.tile([8 * gw, 128], bf16, tag="pef")
                        nc.tensor.transpose(pef[:], GT[g]["EFB"][:], I128)
                        V.tensor_copy(efs[g][:], pef[:])

                # stages 2+3: L1(i-1) + L2(i-2) into one PSUM arena,
                # then one fused silu over both halves
                j1, j2 = i - 1, i - 2
                a1 = 0 <= j1 < NBLK
                a2 = 0 <= j2 < NBLK
                if a1 or a2:
                    AR = par.tile([128, 1024], f32, tag="arena")
                    if a1:
                        e2 = efs[j1 // 2]
                        u = j1 % 2
                        ro = 64 * u
                        for k in range(4):
                            nc.tensor.matmul(
                                AR[:, 128 * k:128 * k + 128],
                                W1P[u][k], e2[ro:ro + 64, :],
                                start=True, stop=True)
                        if u == 1 or j1 == NBLK - 1:
                            del efs[j1 // 2]
                    if a2:
                        nc.tensor.matmul(AR[:, 512:1024], W2BD,
                                         tts[i - 1][:, 0:512],
                                         start=True, stop=True)
                    tts[i] = ttp.tile([128, 1024], bf16, tag="tt", name="tt")
                    lo = 0 if a1 else 512
                    hi = 1024 if a2 else 512
                    nc.scalar.activation(tts[i][:, lo:hi], AR[:, lo:hi],
                                         AF.Silu)

                # stage 4: L3 transposed (PE) + silu3 (ACT) + Gamma
                # product (DVE)
                j = i - 3
                if 0 <= j < NBLK:
                    t2 = tts.pop(i - 1)[:, 512:1024]
                    q3 = pq3.tile([128, 512], f32, tag="q3")
                    for c in range(4):
                        nc.tensor.matmul(
                            q3[:, 128 * c:128 * c + 128],
                            t2[:, 128 * c:128 * c + 128], W3XY,
                            start=True, stop=True)
                    t3e = t3p.tile([128, 512], bf16, tag="t3e", name="t3e")
                    nc.scalar.activation(t3e[:], q3[:], AF.Silu)
                    qss[j] = qp.tile([128, 512], bf16, tag="qs", name="qs")
                    nc.vector.tensor_tensor(qss[j][:], t3e[:], gts[j][:],
                                            OP.mult)
                    del gts[j]

                # stage 5: scatter (PE, N=64) accumulating MSG node tiles
                j = i - 4
                if 0 <= j < NBLK:
                    for k in range(8):
                        s = 8 * j + k
                        nt_ = s // SEG
                        qcol = 128 * (k // 2) + 64 * (k % 2)
                        nc.tensor.matmul(
                            MSG[:, 64 * nt_:64 * nt_ + 64],
                            ohrs[j][:, 128 * k:128 * k + 128],
                            qss[j][:, qcol:qcol + 64],
                            start=(s % SEG == 0), stop=(s % SEG == SEG - 1),
                            skip_group_check=True)
                    del qss[j], ohrs[j]

                for fn in tail.get(i, ()):
                    fn()

            # ---- epilogue: reduce, scale + species constant, DMA out ----
            MSUM = outp.tile([128, 8], f32)
            nc.vector.tensor_reduce(
                MSUM[:], MSG[:].rearrange("p (n h) -> p n h", h=64),
                mybir.AxisListType.X, OP.add)
            OUTT = outp.tile([128, 8], f32)
            nc.vector.scalar_tensor_tensor(
                OUTT[:], MSUM[:], 1.0 / AVG, CNODE, OP.mult, OP.add)
            nc.sync.dma_start(out_d[:], OUTT[:])

    nc.compile()
    return nc


def _host_prep(inputs):
    pos = np.asarray(inputs["positions"], np.float32)
    shifts = np.asarray(inputs["shifts"], np.float32)
    ei = np.asarray(inputs["edge_index"])
    species = np.asarray(inputs["species"]).astype(np.int64)
    ae = np.asarray(inputs["atomic_energies"], np.float32)
    w_embed = np.asarray(inputs["w_embed"], np.float32)
    w_up = np.asarray(inputs["w_up"], np.float32)
    W1 = np.asarray(inputs["W1"], np.float32)
    W2 = np.asarray(inputs["W2"], np.float32)
    W3 = np.asarray(inputs["W3"], np.float32)
    W4 = np.asarray(inputs["W4"], np.float32)
    w_lin = np.asarray(inputs["w_lin"], np.float32)
    w_skip = np.asarray(inputs["w_skip"], np.float32)
    w_sym = np.asarray(inputs["w_sym"], np.float32)
    w_lin2 = np.asarray(inputs["w_lin2"], np.float32)
    w_ro = np.asarray(inputs["w_readout"], np.float32)

    # collapsed weight tables
    hu = w_embed @ w_up                                   # [Z,K]
    alpha = w_lin2[0] @ w_ro                              # [K]
    delta = np.einsum("qk,zk,k->zq", w_lin[0], w_sym[0], alpha)  # [Z,K]
    W4_0 = np.ascontiguousarray(W4.reshape(64, K, 4)[:, :, 0])   # [64,K]
    Gamma = np.einsum("hk,sk,zk->szh", W4_0, hu, delta)   # [Z,Z,64]
    sct = np.einsum("zk,zkj->zj", w_embed, w_skip) / np.sqrt(Z)
    cz = ae + sct @ w_ro                                  # [Z]

    send, recv = ei[0].astype(np.int64), ei[1].astype(np.int64)
    vec = pos[recv] - pos[send] + shifts
    rsq = (vec * vec).sum(-1)
    keep = rsq < (R_MAX * R_MAX + 1e-3)
    rr = np.sqrt(rsq[keep] + 1e-9)   # edge lengths (same +eps as reference)
    sp_s = species[send[keep]]
    recv = recv[keep]
    sp_r = species[recv]

    # degree-balanced global node -> (core, tile, row) packing
    deg_full = np.bincount(recv, minlength=N_NODES)
    NBINS = NCORES * NT
    core_of = np.zeros(N_NODES, np.int64)
    ntile_of = np.zeros(N_NODES, np.int64)
    lrow_of = np.zeros(N_NODES, np.int64)
    order_n = np.argsort(-deg_full, kind="stable")
    loads = np.zeros(NBINS, np.int64)
    fill = np.zeros(NBINS, np.int64)
    for n_ in order_n:
        cand = np.where(fill < 128)[0]
        b = cand[np.argmin(loads[cand])]
        core_of[n_] = b // NT
        ntile_of[n_] = b % NT
        lrow_of[n_] = fill[b]
        fill[b] += 1
        loads[b] += deg_full[n_]

    core = core_of[recv]
    ntile = ntile_of[recv]
    lrow = lrow_of[recv]

    order = np.lexsort((ntile, core))
    rr, sp_s, sp_r, lrow = rr[order], sp_s[order], sp_r[order], lrow[order]
    core, ntile = core[order], ntile[order]
    gid = core * NT + ntile
    counts = np.bincount(gid, minlength=NCORES * NT)
    SEG = int(np.ceil(counts.max() / 128))
    S = NT * SEG
    NBLK = S // 8

    Ra = np.full((NCORES, 128, S), 2.0, np.float32)   # pad r=2 (masked)
    IRa = np.full((NCORES, 128, S), 0.5, np.float32)
    Ga = np.zeros((NCORES, NBLK, 128, 8, 64), BF16)
    OHa = np.zeros((NCORES, NBLK, 128, 8, 128), BF16)

    Gedge = Gamma[sp_s, sp_r].astype(BF16)   # [E, 64]

    starts = np.zeros(NCORES * NT + 1, np.int64)
    np.cumsum(counts, out=starts[1:])
    for c_ in range(NCORES):
        for t in range(NT):
            g = c_ * NT + t
            a, b = starts[g], starts[g + 1]
            n = b - a
            idx = np.arange(n)
            sub = t * SEG + idx // 128        # global subtile
            row = idx % 128
            blk = sub // 8
            jj = sub % 8
            Ra[c_, row, sub] = rr[a:b]
            IRa[c_, row, sub] = 1.0 / rr[a:b]
            Ga[c_, blk, row, jj, :] = Gedge[a:b]
            OHa[c_, blk, row, jj, lrow[a:b]] = 1.0

    cnode = np.zeros((NCORES, 128, 8), np.float32)
    cnode[core_of, lrow_of, ntile_of] = cz[species]

    n_ = np.arange(1, NB + 1, dtype=np.float32)
    cb8 = n_ / (2.0 * R_MAX)    # th = cb*r ; sin(2*pi*th) = sin(n*pi*r/R)
    constf = np.zeros((NCORES, 128, 26), np.float32)
    constf[:, :, 0:8] = cb8[None, None, :]
    constf[:, :, 8:16] = cnode
    constf[:, :, 16] = np.full((1,), 1, np.int32).view(np.float32)[0]
    constf[:, :, 17] = np.full((1,), 0x5F3759DF, np.int32).view(np.float32)[0]
    for k, v in enumerate([0.5, 1.5, 15.0, 21.0, 35.0, 1.0, 1.0 / R_MAX,
                           float(0x5F3759DF)]):
        constf[:, :, 18 + k] = v

    # bf16 weight consts; sqrt(2/R) folded into W1
    W1s = W1 * np.sqrt(2.0 / R_MAX)
    ceb = np.eye(128, dtype=np.float32)
    crb = np.zeros((128, 768), np.float32)
    for j in range(4):           # W1P[j]: pair j of each 8-subtile block
        crb[16 * j:16 * j + 8, 128 * j:128 * j + 64] = W1s
        crb[16 * j + 8:16 * j + 16, 128 * j + 64:128 * j + 128] = W1s
    crb[64:128, 0:512] = crb[0:64, 0:512]   # copy for base-partition 64
    crb[0:64, 512:576] = W2
    crb[64:128, 576:640] = W2
    crb[0:64, 640:704] = W3
    crb[64:128, 704:768] = W3

    SA_W = min(32, S)
    vea = np.ascontiguousarray(np.concatenate(
        [Ra[:, :, 0:SA_W], IRa[:, :, 0:SA_W]], axis=2))
    veb = np.ascontiguousarray(np.concatenate(
        [Ra[:, :, SA_W:], IRa[:, :, SA_W:]], axis=2))

    return (SEG, vea, veb, Ga, OHa, constf, ceb.astype(BF16),
            crb.astype(BF16), core_of, ntile_of, lrow_of)


def kernel(**inputs):
    global LAST_RESULTS
    from concourse.bass_utils import run_bass_kernel_spmd

    (SEG, vea, veb, Ga, OHa, constf, ceb, crb,
     core_of, ntile_of, lrow_of) = _host_prep(inputs)
    S = NT * SEG
    NBLK = S // 8
    if SEG not in _prog_cache:
        _prog_cache[SEG] = _build_program(SEG)
    nc = _prog_cache[SEG]

    in_maps = []
    for c_ in range(NCORES):
        m = {
            "constf": np.ascontiguousarray(constf[c_]),
            "vea": vea[c_],
            "ceb": ceb,
            "veb": veb[c_],
            "crb": crb,
            "gtab": np.ascontiguousarray(Ga[c_].reshape(NBLK, 128, 512)),
            "ohr": np.ascontiguousarray(OHa[c_].reshape(NBLK, 128, 1024)),
        }
        in_maps.append(m)

    res = run_bass_kernel_spmd(
        nc, in_maps, core_ids=list(range(NCORES)), trace=TRACE)
    LAST_RESULTS = res

    Oall = np.stack([res.results[c_]["out"] for c_ in range(NCORES)])
    out = Oall[core_of, lrow_of, ntile_of]
    return out.astype(np.float32)


# revision 51
# speedup vs baseline: 1.2496x; 1.2496x over previous
"""MACE-style GNN message passing on 8 Trainium2 NeuronCores.

Only the l=0 (scalar) channel of the reference reaches the output, so the
network collapses algebraically: per edge, the radial MLP's last hidden
t3 (64) is dotted with a per-(sender-species, receiver-species) vector
Gamma[s,z] = W4_0 @ (hu[s] * delta[z]), where hu = w_embed@w_up and
delta[z] folds w_lin[0], w_sym[0], w_lin2[0] and w_readout.  Node energy
is then ae[z]+beta[z] + (1/16) * scatter_sum(eps_e).

v2 device pipeline (vs v1): geometry split into independent 16-subtile
units with private tiles (U0/U2/U4 on DVE, U1/U3 on the otherwise-idle
Pool engine); act tables preloaded with dummy calls; warm-up matmuls
gated on an early small DMA; silu1+silu2 fused into one [128,1024] ACT
over a 2-bank PSUM arena; L1 uses [16,128] block-diag weights (LDW 16);
per-edge scalar eps = sum_h t3*Gamma computed on DVE so the scatter
matmuls have N=1 and MSG is just [128, 8]; epilogue is one fused STT.

Sharding: receivers range-partitioned via degree-balanced greedy packing
into 64 128-node tiles (8 tiles/core); per (core, node-tile) edge groups
padded to a uniform SEG subtiles of 128 so all cores run one SPMD
program.  Edges with r >= r_max are dropped on host; pad slots are
masked by zero one-hot rows and zero Gamma rows.
"""

import sys
import numpy as np

sys.path.insert(0, "/opt/trn_rl_repo")

import ml_dtypes

BF16 = ml_dtypes.bfloat16

R_MAX = 5.0
AVG = 16.0
N_NODES = 8000
Z = 10
K = 128
NB = 8
NCORES = 8
NPC = N_NODES // NCORES       # nodes per core (1000)
NT = 8                        # node tiles per core (128 nodes each)

SIN_DIRECT = False     # ACT Sin table cannot handle args beyond ~pi
TRACE = False
LAST_RESULTS = None

_prog_cache = {}


def _build_program(SEG):
    """SPMD Bass program; SEG = 128-edge subtiles per 128-node tile."""
    from concourse import bass, bacc, mybir
    from concourse.tile import TileContext
    from contextlib import ExitStack

    f32 = mybir.dt.float32
    bf16 = mybir.dt.bfloat16
    i32 = mybir.dt.int32
    AF = mybir.ActivationFunctionType
    OP = mybir.AluOpType
    PSUM = bass.MemorySpace.PSUM

    S = NT * SEG              # total subtiles per core
    NBLK = S // 8             # 1024-edge blocks
    SA_W = min(32, S)         # subtiles in the early [r,1/r] DMA

    nc = bacc.Bacc(None, target_bir_lowering=False)

    cf_d = nc.dram_tensor("constf", [128, 26], f32, kind="ExternalInput")
    vea_d = nc.dram_tensor("vea", [128, 2 * SA_W], f32, kind="ExternalInput")
    ceb_d = nc.dram_tensor("ceb", [128, 128], bf16, kind="ExternalInput")
    veb_d = nc.dram_tensor("veb", [128, 2 * (S - SA_W)], f32,
                           kind="ExternalInput")
    crb_d = nc.dram_tensor("crb", [128, 768], bf16, kind="ExternalInput")
    g_d = nc.dram_tensor("gtab", [NBLK, 128, 512], bf16, kind="ExternalInput")
    ohr_d = nc.dram_tensor("ohr", [NBLK, 128, 1024], bf16,
                           kind="ExternalInput")
    out_d = nc.dram_tensor("out", [128, 8], f32, kind="ExternalOutput")

    with TileContext(nc) as tc:
        with ExitStack() as stack:
            # one pool per DMA'd tensor: readers of a pool's tile appear to
            # wait on ALL outstanding DMAs into that pool, so sharing a pool
            # serializes consumers behind the slowest DMA
            cpf = stack.enter_context(tc.tile_pool(name="cpf", bufs=1))
            cpe = stack.enter_context(tc.tile_pool(name="cpe", bufs=1))
            cpr = stack.enter_context(tc.tile_pool(name="cpr", bufs=1))
            vpa = stack.enter_context(tc.tile_pool(name="vpa", bufs=1))
            vpb = stack.enter_context(tc.tile_pool(name="vpb", bufs=1))
            cp = stack.enter_context(tc.tile_pool(name="const", bufs=1))
            geo = stack.enter_context(tc.tile_pool(name="geo", bufs=1))
            efsp = stack.enter_context(tc.tile_pool(name="efsp", bufs=3))
            gp = stack.enter_context(tc.tile_pool(name="gp", bufs=5))
            ohp = stack.enter_context(tc.tile_pool(name="ohp", bufs=5))
            ttp = stack.enter_context(tc.tile_pool(name="ttp", bufs=3))
            t3p = stack.enter_context(tc.tile_pool(name="t3p", bufs=3))
            qp = stack.enter_context(tc.tile_pool(name="qp", bufs=3))
            epp = stack.enter_context(tc.tile_pool(name="epp", bufs=3))
            outp = stack.enter_context(tc.tile_pool(name="outp", bufs=1))
            pefp = stack.enter_context(tc.tile_pool(name="pefp", bufs=2,
                                                    space=PSUM))
            par = stack.enter_context(tc.tile_pool(name="par", bufs=2,
                                                   space=PSUM))
            pq3 = stack.enter_context(tc.tile_pool(name="pq3", bufs=1,
                                                   space=PSUM))
            pmsg = stack.enter_context(tc.tile_pool(name="pmsg", bufs=1,
                                                    space=PSUM))

            # ---- constants: smallest / most critical DMAs first ----
            CTF = cpf.tile([128, 26], f32)
            nc.sync.dma_start(CTF[:], cf_d[:], single_packet=True)
            VEA = vpa.tile([128, 2 * SA_W], f32, name="VEA")
            nc.sync.dma_start(VEA[:], vea_d[:])
            CEB = cpe.tile([128, 128], bf16)
            nc.sync.dma_start(CEB[:], ceb_d[:])
            VEB = vpb.tile([128, 2 * (S - SA_W)], f32, name="VEB")
            nc.sync.dma_start(VEB[:], veb_d[:])
            CRB = cpr.tile([128, 768], bf16)
            nc.sync.dma_start(CRB[:], crb_d[:])

            CB8 = CTF[:, 0:8]
            CNODE = CTF[:, 8:16]
            ONEI = CTF[:, 16:17].bitcast(i32)
            MAGIC = CTF[:, 17:18].bitcast(i32)
            MAGICF = float(0x5F3759DF)
            CCOL = {v: CTF[:, 18 + k:19 + k] for k, v in enumerate(
                [0.5, 1.5, 15.0, 21.0, 35.0, 1.0, 1.0 / R_MAX, MAGICF])}
            I128 = CEB[:, 0:128]
            # W1P duplicated in both partition halves so the lhsT base
            # partition can match the rhs (efs half) base partition
            W1P = [[CRB[ro:ro + 64, 128 * j:128 * j + 128]
                    for j in range(4)] for ro in (0, 64)]
            W2BD = CRB[:, 512:640]
            W3XY = CRB[:, 640:768]

            tc.strict_bb_all_engine_barrier()

            # ---- ACT table preload: dummy Sin + Silu on scratch ----
            SCR = cp.tile([128, 1], f32)
            nc.gpsimd.memset(SCR[:], 0.25)
            DS = cp.tile([128, 1], f32)
            nc.scalar.activation(DS[:], SCR[:], AF.Sin, scale=1.0)
            nc.scalar.activation(DS[:], SCR[:], AF.Silu)

            # ---- PE pstate warm-up: small matmuls gated on CEB only ----
            WUP = pq3.tile([128, 512], f32, tag="q3")
            for _ in range(5):
                nc.tensor.matmul(WUP[:, 0:128], I128, I128,
                                 start=True, stop=True, skip_group_check=True)

            # ---- geometry, all on DVE.  Host supplies per-edge [r, 1/r]
            # (edge lengths; already computed host-side for the r<R filter).
            # Device computes the cutoff envelope, bessel phases, sin and
            # the ef features.  (Pool proved ~5x slower per op on hw.)
            V = nc.vector
            P = nc.gpsimd
            SC = geo.tile([128, 4 * S], f32, name="SC")

            def rsl(s0, s1):
                """(r, inv_r) slices for subtile-cols [s0, s1)."""
                if s1 <= SA_W:
                    return (VEA[:, s0:s1], VEA[:, SA_W + s0:SA_W + s1])
                o = s0 - SA_W
                return (VEB[:, o:o + (s1 - s0)],
                        VEB[:, (S - SA_W) + o:(S - SA_W) + o + (s1 - s0)])

            def emit_scalars(s0, s1):
                """cutoff envelope env(r)/r for subtile-cols [s0, s1)."""
                def sl(i):
                    return SC[:, i * S + s0:i * S + s1]

                x, u1, u2, wv = (sl(i) for i in range(4))
                r_, ir_ = rsl(s0, s1)
                V.tensor_scalar(x, r_, 1.0 / R_MAX, None, OP.mult)
                V.tensor_tensor(u1, x, x, OP.mult)
                V.tensor_tensor(u1, u1, u1, OP.mult)
                V.tensor_tensor(u1, u1, x, OP.mult)      # x^5
                V.tensor_scalar(u2, x, -15.0, 35.0, OP.mult, OP.add)
                V.tensor_tensor(u2, u2, x, OP.mult)
                V.scalar_tensor_tensor(u1, u2, -21.0, u1, OP.add, OP.mult)
                V.scalar_tensor_tensor(wv, u1, 1.0, ir_, OP.add, OP.mult)
                # wv = env(r)/r  (sqrt(2/R) folded into W1)

            # bessel groups: 16-subtile strides (2 blocks per group, so the
            # ef transpose can be one [128,128] DMA-xbar per group)
            GBOUND = list(range(0, S, 16)) + [S]
            NG = len(GBOUND) - 1
            GT = {}

            def emit_bessel(g):
                s0, s1 = GBOUND[g], GBOUND[g + 1]
                w = s1 - s0
                t = {
                    "TH": geo.tile([128, 8 * w], f32, name=f"TH{g}"),
                    "SH": geo.tile([128, 8 * w], f32, name=f"SH{g}"),
                    "EFB": geo.tile([128, 8 * w], bf16, name=f"EFB{g}"),
                }
                GT[g] = t
                r_, _ = rsl(s0, s1)
                wenv = SC[:, 3 * S + s0:3 * S + s1]
                V.tensor_tensor(
                    t["TH"][:].rearrange("p (s b) -> p s b", b=8),
                    CB8.unsqueeze(1).broadcast_to([128, w, 8]),
                    r_.unsqueeze(2).broadcast_to([128, w, 8]),
                    OP.mult)
                ki = geo.tile([128, 8 * w], i32, name=f"KI{g}")
                kf = geo.tile([128, 8 * w], f32, name=f"KF{g}")
                sa = geo.tile([128, 8 * w], f32, name=f"SA{g}")
                V.tensor_copy(ki[:], t["TH"][:])
                V.tensor_copy(kf[:], ki[:])
                V.tensor_tensor(sa[:], t["TH"][:], kf[:], OP.subtract)
                nc.scalar.activation(t["SH"][:], sa[:], AF.Sin,
                                     scale=float(2 * np.pi))
                V.tensor_tensor(
                    t["EFB"][:].rearrange("p (s b) -> p s b", b=8),
                    t["SH"][:].rearrange("p (s b) -> p s b", b=8),
                    wenv.unsqueeze(2).broadcast_to([128, w, 8]),
                    OP.mult)

            def efb_col(i):
                """EFB access for block i: (group tile, col offset)."""
                g = (8 * i) // 16
                off = 8 * (8 * i - GBOUND[g])
                return GT[g]["EFB"], off

            # chains: [0:SA_W] pre, rest @iter0; bessel g0 pre, g @iter g-1
            emit_scalars(0, min(SA_W, S))
            emit_bessel(0)

            tail = {}

            def add_tail(it, fn):
                tail.setdefault(it, []).append(fn)

            for g in range(1, NG):
                add_tail(g - 1, (lambda gg: lambda: emit_bessel(gg))(g))
            if S > SA_W:
                # after bessel1 in tail[0]: bessel1 only needs chain A
                add_tail(0, lambda: emit_scalars(SA_W, S))

            # ---- software-pipelined block loop ----
            # stage skew: efT(i) -> L1(i-1) -> L2(i-2) + fused silu12
            #   -> L3T(i-3)+silu3+qss -> scatter(i-4)
            MSG = pmsg.tile([128, 512], f32, tag="msg")
            efs = {}
            tts = {}   # per-iter arena: [0:512]=t1(i-1), [512:1024]=t2(i-2)
            t3s = {}
            qss = {}
            gts = {}
            ohrs = {}
            NITER = NBLK + 4
            for i in range(NITER):
                if i < NBLK:
                    gts[i] = gp.tile([128, 512], bf16, tag="gt", name="gt")
                    nc.sync.dma_start(gts[i][:], g_d[i])
                    ohrs[i] = ohp.tile([128, 1024], bf16, tag="ohr",
                                       name="ohrt")
                    nc.sync.dma_start(ohrs[i][:], ohr_d[i])

                # stage 1: one [128,128] ef transpose per 2-block group
                # (PE) + copy to SBUF (DVE)
                if i < NBLK and i % 2 == 0:
                    g = i // 2
                    gw = GBOUND[g + 1] - GBOUND[g]
                    efs[g] = efsp.tile([8 * gw, 128], bf16, tag="efs",
                                       name="efs")
                    pef = pefp.tile([8 * gw, 128], bf16, tag="pef")
                    nc.tensor.transpose(pef[:], GT[g]["EFB"][:], I128)
                    V.tensor_copy(efs[g][:], pef[:])

                # stages 2+3: L1(i-1) + L2(i-2) into one PSUM arena,
                # then one fused silu over both halves
                j1, j2 = i - 1, i - 2
                a1 = 0 <= j1 < NBLK
                a2 = 0 <= j2 < NBLK
                if a1 or a2:
                    AR = par.tile([128, 1024], f32, tag="arena")
                    if a1:
                        e2 = efs[j1 // 2]
                        u = j1 % 2
                        ro = 64 * u
                        for k in range(4):
                            nc.tensor.matmul(
                                AR[:, 128 * k:128 * k + 128],
                                W1P[u][k], e2[ro:ro + 64, :],
                                start=True, stop=True)
                        if u == 1 or j1 == NBLK - 1:
                            del efs[j1 // 2]
                    if a2:
                        nc.tensor.matmul(AR[:, 512:1024], W2BD,
                                         tts[i - 1][:, 0:512],
                                         start=True, stop=True)
                    tts[i] = ttp.tile([128, 1024], bf16, tag="tt", name="tt")
                    lo = 0 if a1 else 512
                    hi = 1024 if a2 else 512
                    nc.scalar.activation(tts[i][:, lo:hi], AR[:, lo:hi],
                                         AF.Silu)

                # stage 4: L3 transposed (PE) + silu3 (ACT) + Gamma
                # product (DVE)
                j = i - 3
                if 0 <= j < NBLK:
                    t2 = tts.pop(i - 1)[:, 512:1024]
                    q3 = pq3.tile([128, 512], f32, tag="q3")
                    for c in range(4):
                        nc.tensor.matmul(
                            q3[:, 128 * c:128 * c + 128],
                            t2[:, 128 * c:128 * c + 128], W3XY,
                            start=True, stop=True)
                    t3e = t3p.tile([128, 512], bf16, tag="t3e", name="t3e")
                    nc.scalar.activation(t3e[:], q3[:], AF.Silu)
                    qss[j] = qp.tile([128, 512], bf16, tag="qs", name="qs")
                    nc.vector.tensor_tensor(qss[j][:], t3e[:], gts[j][:],
                                            OP.mult)
                    del gts[j]

                # stage 5: scatter (PE, N=64) accumulating MSG node tiles
                j = i - 4
                if 0 <= j < NBLK:
                    for k in range(8):
                        s = 8 * j + k
                        nt_ = s // SEG
                        qcol = 128 * (k // 2) + 64 * (k % 2)
                        nc.tensor.matmul(
                            MSG[:, 64 * nt_:64 * nt_ + 64],
                            ohrs[j][:, 128 * k:128 * k + 128],
                            qss[j][:, qcol:qcol + 64],
                            start=(s % SEG == 0), stop=(s % SEG == SEG - 1),
                            skip_group_check=True)
                    del qss[j], ohrs[j]

                for fn in tail.get(i, ()):
                    fn()

            # ---- epilogue: reduce, scale + species constant, DMA out ----
            MSUM = outp.tile([128, 8], f32)
            nc.vector.tensor_reduce(
                MSUM[:], MSG[:].rearrange("p (n h) -> p n h", h=64),
                mybir.AxisListType.X, OP.add)
            OUTT = outp.tile([128, 8], f32)
            nc.vector.scalar_tensor_tensor(
                OUTT[:], MSUM[:], 1.0 / AVG, CNODE, OP.mult, OP.add)
            nc.sync.dma_start(out_d[:], OUTT[:])

    nc.compile()
    return nc


def _host_prep(inputs):
    pos = np.asarray(inputs["positions"], np.float32)
    shifts = np.asarray(inputs["shifts"], np.float32)
    ei = np.asarray(inputs["edge_index"])
    species = np.asarray(inputs["species"]).astype(np.int64)
    ae = np.asarray(inputs["atomic_energies"], np.float32)
    w_embed = np.asarray(inputs["w_embed"], np.float32)
    w_up = np.asarray(inputs["w_up"], np.float32)
    W1 = np.asarray(inputs["W1"], np.float32)
    W2 = np.asarray(inputs["W2"], np.float32)
    W3 = np.asarray(inputs["W3"], np.float32)
    W4 = np.asarray(inputs["W4"], np.float32)
    w_lin = np.asarray(inputs["w_lin"], np.float32)
    w_skip = np.asarray(inputs["w_skip"], np.float32)
    w_sym = np.asarray(inputs["w_sym"], np.float32)
    w_lin2 = np.asarray(inputs["w_lin2"], np.float32)
    w_ro = np.asarray(inputs["w_readout"], np.float32)

    # collapsed weight tables
    hu = w_embed @ w_up                                   # [Z,K]
    alpha = w_lin2[0] @ w_ro                              # [K]
    delta = np.einsum("qk,zk,k->zq", w_lin[0], w_sym[0], alpha)  # [Z,K]
    W4_0 = np.ascontiguousarray(W4.reshape(64, K, 4)[:, :, 0])   # [64,K]
    Gamma = np.einsum("hk,sk,zk->szh", W4_0, hu, delta)   # [Z,Z,64]
    sct = np.einsum("zk,zkj->zj", w_embed, w_skip) / np.sqrt(Z)
    cz = ae + sct @ w_ro                                  # [Z]

    send, recv = ei[0].astype(np.int64), ei[1].astype(np.int64)
    vec = pos[recv] - pos[send] + shifts
    rsq = (vec * vec).sum(-1)
    keep = rsq < (R_MAX * R_MAX + 1e-3)
    rr = np.sqrt(rsq[keep] + 1e-9)   # edge lengths (same +eps as reference)
    sp_s = species[send[keep]]
    recv = recv[keep]
    sp_r = species[recv]

    # degree-balanced global node -> (core, tile, row) packing
    deg_full = np.bincount(recv, minlength=N_NODES)
    NBINS = NCORES * NT
    core_of = np.zeros(N_NODES, np.int64)
    ntile_of = np.zeros(N_NODES, np.int64)
    lrow_of = np.zeros(N_NODES, np.int64)
    order_n = np.argsort(-deg_full, kind="stable")
    loads = np.zeros(NBINS, np.int64)
    fill = np.zeros(NBINS, np.int64)
    for n_ in order_n:
        cand = np.where(fill < 128)[0]
        b = cand[np.argmin(loads[cand])]
        core_of[n_] = b // NT
        ntile_of[n_] = b % NT
        lrow_of[n_] = fill[b]
        fill[b] += 1
        loads[b] += deg_full[n_]

    core = core_of[recv]
    ntile = ntile_of[recv]
    lrow = lrow_of[recv]

    order = np.lexsort((ntile, core))
    rr, sp_s, sp_r, lrow = rr[order], sp_s[order], sp_r[order], lrow[order]
    core, ntile = core[order], ntile[order]
    gid = core * NT + ntile
    counts = np.bincount(gid, minlength=NCORES * NT)
    SEG = int(np.ceil(counts.max() / 128))
    S = NT * SEG
    NBLK = S // 8

    Ra = np.full((NCORES, 128, S), 2.0, np.float32)   # pad r=2 (masked)
    IRa = np.full((NCORES, 128, S), 0.5, np.float32)
    Ga = np.zeros((NCORES, NBLK, 128, 8, 64), BF16)
    OHa = np.zeros((NCORES, NBLK, 128, 8, 128), BF16)

    Gedge = Gamma[sp_s, sp_r].astype(BF16)   # [E, 64]

    starts = np.zeros(NCORES * NT + 1, np.int64)
    np.cumsum(counts, out=starts[1:])
    for c_ in range(NCORES):
        for t in range(NT):
            g = c_ * NT + t
            a, b = starts[g], starts[g + 1]
            n = b - a
            idx = np.arange(n)
            sub = t * SEG + idx // 128        # global subtile
            row = idx % 128
            blk = sub // 8
            jj = sub % 8
            Ra[c_, row, sub] = rr[a:b]
            IRa[c_, row, sub] = 1.0 / rr[a:b]
            Ga[c_, blk, row, jj, :] = Gedge[a:b]
            OHa[c_, blk, row, jj, lrow[a:b]] = 1.0

    cnode = np.zeros((NCORES, 128, 8), np.float32)
    cnode[core_of, lrow_of, ntile_of] = cz[species]

    n_ = np.arange(1, NB + 1, dtype=np.float32)
    cb8 = n_ / (2.0 * R_MAX)    # th = cb*r ; sin(2*pi*th) = sin(n*pi*r/R)
    constf = np.zeros((NCORES, 128, 26), np.float32)
    constf[:, :, 0:8] = cb8[None, None, :]
    constf[:, :, 8:16] = cnode
    constf[:, :, 16] = np.full((1,), 1, np.int32).view(np.float32)[0]
    constf[:, :, 17] = np.full((1,), 0x5F3759DF, np.int32).view(np.float32)[0]
    for k, v in enumerate([0.5, 1.5, 15.0, 21.0, 35.0, 1.0, 1.0 / R_MAX,
                           float(0x5F3759DF)]):
        constf[:, :, 18 + k] = v

    # bf16 weight consts; sqrt(2/R) folded into W1
    W1s = W1 * np.sqrt(2.0 / R_MAX)
    ceb = np.eye(128, dtype=np.float32)
    crb = np.zeros((128, 768), np.float32)
    for j in range(4):           # W1P[j]: pair j of each 8-subtile block
        crb[16 * j:16 * j + 8, 128 * j:128 * j + 64] = W1s
        crb[16 * j + 8:16 * j + 16, 128 * j + 64:128 * j + 128] = W1s
    crb[64:128, 0:512] = crb[0:64, 0:512]   # copy for base-partition 64
    crb[0:64, 512:576] = W2
    crb[64:128, 576:640] = W2
    crb[0:64, 640:704] = W3
    crb[64:128, 704:768] = W3

    SA_W = min(32, S)
    vea = np.ascontiguousarray(np.concatenate(
        [Ra[:, :, 0:SA_W], IRa[:, :, 0:SA_W]], axis=2))
    veb = np.ascontiguousarray(np.concatenate(
        [Ra[:, :, SA_W:], IRa[:, :, SA_W:]], axis=2))

    return (SEG, vea, veb, Ga, OHa, constf, ceb.astype(BF16),
            crb.astype(BF16), core_of, ntile_of, lrow_of)


def kernel(**inputs):
    global LAST_RESULTS
    from concourse.bass_utils import run_bass_kernel_spmd

    (SEG, vea, veb, Ga, OHa, constf, ceb, crb,
     core_of, ntile_of, lrow_of) = _host_prep(inputs)
    S = NT * SEG
    NBLK = S // 8
    if SEG not in _prog_cache:
        _prog_cache[SEG] = _build_program(SEG)
    nc = _prog_cache[SEG]

    in_maps = []
    for c_ in range(NCORES):
        m = {
            "constf": np.ascontiguousarray(constf[c_]),
            "vea": vea[c_],
            "ceb": ceb,
            "veb": veb[c_],
            "crb": crb,
            "gtab": np.ascontiguousarray(Ga[c_].reshape(NBLK, 128, 512)),
            "ohr": np.ascontiguousarray(OHa[c_].reshape(NBLK, 128, 1024)),
        }
        in_maps.append(m)

    res = run_bass_kernel_spmd(
        nc, in_maps, core_ids=list(range(NCORES)), trace=TRACE)
    LAST_RESULTS = res

    Oall = np.stack([res.results[c_]["out"] for c_ in range(NCORES)])
    out = Oall[core_of, lrow_of, ntile_of]
    return out.astype(np.float32)
